# revision 3
# baseline (speedup 1.0000x reference)
"""AnisotropicEdgeFilter Trainium2 kernel (8 NeuronCores, data-parallel over edges).

Math (per edge e):
    h  = elu(pos @ W1 + b1)                       [E, 128]
    ew = (h @ W2 + b2).reshape(E, 8, 32)          per-edge filter
    out[e, o] = sum_i attr[e, i] * ew[e, i, o]    [E, 32]

Device-side restructuring:
    g = elu(x) + 1 = relu(x) + min(exp(x), 1)     (x = pos@W1+b1, b1 folded via
                                                   ones-row augmentation of pos/W1)
    ew + b2 = g @ W2 + b2'        with b2' = b2 - W2.sum(0)   (the "-1" fold)
    out = sum_i attr_i * (g @ W2)_i  + attr @ reshape(b2', (8,32))
          ^ on-device einsum           ^ "abias", precomputed on host

Layouts: hT [hidden=partition, edge=free] so the ELU'd activations are directly
the stationary weights of the W2 matmul; einsum runs in [edge=partition] layout
on VectorE as a 3D-broadcast multiply + innermost reduce.
"""

import os
import sys

import numpy as np

sys.path.insert(0, "/opt/trn_rl_repo")

import ml_dtypes  # noqa: E402

E = 500000
IN_SIZE = 8
POS_SIZE = 6
HIDDEN = 128
OUT_SIZE = 32
N_CORES = 8
CHUNK = 512            # edges per inner chunk (4 sub-tiles of 128)
N_CHUNKS = 123
E_LOC = CHUNK * N_CHUNKS      # 62976 edges per core
E_PAD = E_LOC * N_CORES       # 503808

_BF16 = ml_dtypes.bfloat16

_COMPILED = {}


def _build_nc():
    import concourse.bass as bass
    import concourse.tile as tile
    from concourse import bacc, mybir

    dt = mybir.dt
    nc = bacc.Bacc(
        "TRN2",
        target_bir_lowering=False,
        debug=False,
        num_devices=N_CORES,
    )

    post_d = nc.dram_tensor("post", [POS_SIZE + 1, E_LOC], dt.bfloat16, kind="ExternalInput")
    attr_d = nc.dram_tensor("attr", [N_CHUNKS, 128, 4, IN_SIZE], dt.bfloat16, kind="ExternalInput")
    abias_d = nc.dram_tensor("abias", [N_CHUNKS, 128, 4, OUT_SIZE], dt.bfloat16, kind="ExternalInput")
    w1_d = nc.dram_tensor("w1aug", [POS_SIZE + 1, HIDDEN], dt.bfloat16, kind="ExternalInput")
    w2_d = nc.dram_tensor("w2", [HIDDEN, IN_SIZE * OUT_SIZE], dt.bfloat16, kind="ExternalInput")
    out_d = nc.dram_tensor("out", [N_CHUNKS, 128, 4, OUT_SIZE], dt.bfloat16, kind="ExternalOutput")

    ACT = mybir.ActivationFunctionType
    ALU = mybir.AluOpType

    with tile.TileContext(nc) as tc:
        with (
            tc.tile_pool(name="wpool", bufs=1) as wpool,
            tc.tile_pool(name="inpool", bufs=3) as inpool,
            tc.tile_pool(name="hps", bufs=2, space="PSUM") as hps_pool,
            tc.tile_pool(name="ewps", bufs=4, space="PSUM") as ewps_pool,
            tc.tile_pool(name="work", bufs=3) as work,
            tc.tile_pool(name="outp", bufs=3) as outp,
        ):
            w1_sb = wpool.tile([POS_SIZE + 1, HIDDEN], dt.bfloat16)
            nc.sync.dma_start(w1_sb[:], w1_d.ap())
            w2_sb = wpool.tile([HIDDEN, IN_SIZE * OUT_SIZE], dt.bfloat16)
            nc.sync.dma_start(w2_sb[:], w2_d.ap())

            post_ap = post_d.ap()
            attr_ap = attr_d.ap()
            abias_ap = abias_d.ap()
            out_ap = out_d.ap()

            for c in range(N_CHUNKS):
                pos_sb = inpool.tile([POS_SIZE + 1, CHUNK], dt.bfloat16, tag="pos")
                nc.sync.dma_start(pos_sb[:], post_ap[:, c * CHUNK : (c + 1) * CHUNK])
                attr_sb = inpool.tile([128, 4, IN_SIZE], dt.bfloat16, tag="attr")
                nc.sync.dma_start(attr_sb[:], attr_ap[c])
                abias_sb = inpool.tile([128, 4, OUT_SIZE], dt.bfloat16, tag="abias")
                nc.sync.dma_start(abias_sb[:], abias_ap[c])

                # x = posT_aug.T @ W1aug  ->  hT psum [hidden=128, CHUNK]
                hps = hps_pool.tile([HIDDEN, CHUNK], dt.float32)
                nc.tensor.matmul(hps[:], w1_sb[:], pos_sb[:], start=True, stop=True)

                # g = relu(x) + min(exp(x), 1)   (= elu(x) + 1)
                e_sb = work.tile([HIDDEN, CHUNK], dt.bfloat16, tag="exp")
                nc.scalar.activation(e_sb[:], hps[:], ACT.Exp)
                v_sb = work.tile([HIDDEN, CHUNK], dt.bfloat16, tag="vmin")
                nc.gpsimd.tensor_scalar_min(v_sb[:], e_sb[:], 1.0)
                g_sb = work.tile([HIDDEN, CHUNK], dt.bfloat16, tag="g")
                nc.vector.scalar_tensor_tensor(
                    g_sb[:], hps[:], 0.0, v_sb[:], op0=ALU.max, op1=ALU.add
                )

                outt = outp.tile([128, 4, OUT_SIZE], dt.bfloat16, tag="outt")
                for s in range(4):
                    # ew = gT.T @ W2 -> [128 edges, 256]
                    ewp = ewps_pool.tile([128, IN_SIZE * OUT_SIZE], dt.float32)
                    nc.tensor.matmul(
                        ewp[:],
                        g_sb[:, s * 128 : (s + 1) * 128],
                        w2_sb[:],
                        start=True,
                        stop=True,
                    )
                    # prod[p, o, i] = ew[p, i*32+o] * attr[p, s, i]
                    prod = work.tile([128, OUT_SIZE, IN_SIZE], dt.bfloat16, tag="prod")
                    ew_v = ewp[:].rearrange("p (i o) -> p o i", i=IN_SIZE, o=OUT_SIZE)
                    at_v = attr_sb[:, s, :].unsqueeze(1).broadcast_to([128, OUT_SIZE, IN_SIZE])
                    nc.vector.tensor_tensor(prod[:], ew_v, at_v, op=ALU.mult)
                    # red[p, o] = sum_i prod[p, o, i]
                    red = work.tile([128, OUT_SIZE], dt.bfloat16, tag="red")
                    with nc.allow_low_precision(reason="8-term bf16 reduce, 2e-2 gate"):
                        nc.vector.tensor_reduce(
                            red[:], prod[:], axis=mybir.AxisListType.X, op=ALU.add
                        )
                    nc.vector.tensor_add(outt[:, s, :], red[:], abias_sb[:, s, :])

                nc.sync.dma_start(out_ap[c], outt[:])

    nc.compile()
    return nc


def _get_compiled():
    if "nc" not in _COMPILED:
        _COMPILED["nc"] = _build_nc()
    return _COMPILED["nc"]


def _prep_shards(edge_attr, edge_pos, W1, b1, W2, b2):
    """Host-side prep: pad, fold biases, transpose, tile, cast to bf16."""
    ea = np.asarray(edge_attr, dtype=np.float32)
    ep = np.asarray(edge_pos, dtype=np.float32)
    W1 = np.asarray(W1, dtype=np.float32)
    b1 = np.asarray(b1, dtype=np.float32)
    W2 = np.asarray(W2, dtype=np.float32)
    b2 = np.asarray(b2, dtype=np.float32)

    n = ea.shape[0]
    pad = E_PAD - n
    ea_p = np.pad(ea, ((0, pad), (0, 0)))
    ep_p = np.pad(ep, ((0, pad), (0, 0)))

    # b2' = b2 - W2.sum(0); abias = attr @ reshape(b2', (8, 32))
    b2p = b2 - W2.sum(axis=0)
    abias = ea_p @ b2p.reshape(IN_SIZE, OUT_SIZE)  # [E_PAD, 32] f32

    w1aug = np.concatenate([W1, b1[None, :]], axis=0).astype(_BF16)  # [7, 128]
    w2_bf = W2.astype(_BF16)

    in_maps = []
    for i in range(N_CORES):
        sl = slice(i * E_LOC, (i + 1) * E_LOC)
        pos_sh = ep_p[sl]  # [E_LOC, 6]
        post = np.empty((POS_SIZE + 1, E_LOC), dtype=_BF16)
        post[:POS_SIZE] = pos_sh.T.astype(_BF16)
        post[POS_SIZE] = _BF16(1.0)
        attr_sh = (
            ea_p[sl].reshape(N_CHUNKS, 4, 128, IN_SIZE).transpose(0, 2, 1, 3)
        ).astype(_BF16)  # [C, 128, 4, 8]
        abias_sh = (
            abias[sl].reshape(N_CHUNKS, 4, 128, OUT_SIZE).transpose(0, 2, 1, 3)
        ).astype(_BF16)  # [C, 128, 4, 32]
        in_maps.append(
            {
                "post": np.ascontiguousarray(post),
                "attr": np.ascontiguousarray(attr_sh),
                "abias": np.ascontiguousarray(abias_sh),
                "w1aug": w1aug,
                "w2": w2_bf,
            }
        )
    return in_maps


def kernel(**inputs) -> np.ndarray:
    from concourse.bass_utils import run_bass_kernel_spmd

    n = inputs["edge_attr"].shape[0]
    in_maps = _prep_shards(
        inputs["edge_attr"], inputs["edge_pos"],
        inputs["W1"], inputs["b1"], inputs["W2"], inputs["b2"],
    )
    nc = _get_compiled()
    res = run_bass_kernel_spmd(nc, in_maps, core_ids=list(range(N_CORES)))
    outs = []
    for i in range(N_CORES):
        o = np.asarray(res.results[i]["out"])  # [C, 128, 4, 32] bf16
        o = o.astype(np.float32).transpose(0, 2, 1, 3).reshape(E_LOC, OUT_SIZE)
        outs.append(o)
    full = np.concatenate(outs, axis=0)[:n]
    return np.ascontiguousarray(full)
